# revision 1
# baseline (speedup 1.0000x reference)
"""Dynamic depthwise 3x3 conv (per-pixel weights) on 8 TRN2 NeuronCores.

out[n,c,y,x] = sum_{ki,kj} xpad[n,c,y+ki-1,x+kj-1] * w[n, c*9+3*ki+kj, y, x]

Sharding: pure data parallel over N=8 (one image per core).
Per-core layout: C=128 on partitions, spatial on the free dim, H processed
in row blocks of R rows. The x block is stored with row stride W+1=129 so
that the single zero element between consecutive rows serves as both the
right-pad of row r and the left-pad of row r+1; all nine taps then read
full-width shifted 3D APs with no boundary special-casing.

The two x buffers are persistent (allocated once, ping-ponged across
blocks): their zero gap columns are written once at kernel start and
survive, because the row DMAs only ever write row interiors.

Work split per block: DVE owns taps 0-5 and their partial-sum chain,
GpSimd owns taps 6-8 and its chain; the only cross-engine dependency is
the final combine on DVE. Loads (x, w) issue from SP's HWDGE queue;
stores issue from ACT's queue so a result-dependent store can never
head-of-line-block the next block's loads.
"""

import numpy as np

import concourse.bass as bass
import concourse.bacc as bacc
import concourse.mybir as mybir
from concourse import tile

N, C, H, W = 8, 128, 128, 128
R = 16  # rows per block
NBLOCKS = H // R
RS = W + 1  # row stride inside the x tile (shared zero gap col)
FREE_X = (R + 2) * RS + 2  # leading zero + R+2 rows + tail slack for tap APs
F32 = mybir.dt.float32
MULT = mybir.AluOpType.mult
ADD = mybir.AluOpType.add
W_GROUP = 1  # 1: one DMA per weight tap; 3: one DMA per 3-tap group


def _rows3d(ap, start, nrows):
    """[128, nrows, 128] view of an x tile at free-offset `start`, row stride RS."""
    return ap[:, start : start + nrows * RS].rearrange("p (r c) -> p r c", c=RS)[
        :, :, 0:W
    ]


def _emit_block(nc, pools, x_t, x_d, w_d, o_d, y0, rb, x_dma_rows):
    wpool, ppool, spool = pools
    lo, hi, slot0 = x_dma_rows
    nc.sync.dma_start(
        out=_rows3d(x_t, 1 + slot0 * RS, hi - lo + 1),
        in_=x_d[:, lo : hi + 1, :],
    )

    if W_GROUP == 1:
        w_ts = []
        for k in range(9):
            w_t = wpool.tile([C, R, W], F32, tag="w", name=f"w_{y0}_{k}")
            nc.sync.dma_start(out=w_t[:, 0:rb, :], in_=w_d[:, k, y0 : y0 + rb, :])
            w_ts.append(w_t)
    else:
        # one DMA per 3-tap group; group g = taps 3g..3g+2 (group 2 = Pool's)
        w_ts = []
        for g in range(3):
            w_g = wpool.tile([C, 3, R, W], F32, tag="w", name=f"w_{y0}_g{g}")
            nc.sync.dma_start(
                out=w_g[:, :, 0:rb, :], in_=w_d[:, 3 * g : 3 * g + 3, y0 : y0 + rb, :]
            )
            w_ts.extend(w_g[:, j] for j in range(3))

    # Product/sum tiles use a [C, R, RS] layout and compute on [:, :, 0:W]:
    # the row stride RS=W+1 makes the access patterns non-mergeable, which
    # empirically runs ~1.6x faster on DVE and ~2x on GpSimd than APs the
    # optimizer merges into one long contiguous run.
    def mul(eng, k):
        ki, kj = divmod(k, 3)
        p_t = ppool.tile([C, R, RS], F32, tag="p", name=f"p_{y0}_{k}")
        eng.tensor_tensor(
            out=p_t[:, 0:rb, 0:W],
            in0=_rows3d(x_t, ki * RS + kj, rb),
            in1=w_ts[k][:, 0:rb, :],
            op=MULT,
        )
        return p_t

    def add(eng, nm, a, b):
        s_t = spool.tile([C, R, RS], F32, tag="s", name=f"{nm}_{y0}")
        eng.tensor_tensor(
            out=s_t[:, 0:rb, 0:W],
            in0=a[:, 0:rb, 0:W],
            in1=b[:, 0:rb, 0:W],
            op=ADD,
        )
        return s_t

    v, g = nc.vector, nc.gpsimd
    # Pool chain: taps 6-8 (independent of DVE)
    p6, p7 = mul(g, 6), mul(g, 7)
    b0 = add(g, "b0", p6, p7)
    p8 = mul(g, 8)
    b1 = add(g, "b1", b0, p8)
    # DVE chain: taps 0-5, then the single cross-engine combine
    p0, p1 = mul(v, 0), mul(v, 1)
    a01 = add(v, "a01", p0, p1)
    p2, p3 = mul(v, 2), mul(v, 3)
    a23 = add(v, "a23", p2, p3)
    p4, p5 = mul(v, 4), mul(v, 5)
    a45 = add(v, "a45", p4, p5)
    a0123 = add(v, "a0123", a01, a23)
    aL = add(v, "aL", a0123, a45)
    o_t = add(v, "o", aL, b1)
    # store on ACT's HWDGE queue
    nc.scalar.dma_start(out=o_d[:, y0 : y0 + rb, :], in_=o_t[:, 0:rb, 0:W])


def build_nc(repeat=1, bufs=(11, 7, 5)):
    nc = bacc.Bacc("TRN2", target_bir_lowering=False, debug=False)
    x_d = nc.dram_tensor("x", [C, H, W], F32, kind="ExternalInput")
    w_d = nc.dram_tensor("w", [C, 9, H, W], F32, kind="ExternalInput")
    o_d = nc.dram_tensor("out", [C, H, W], F32, kind="ExternalOutput")
    with tile.TileContext(nc) as tc:
        with (
            tc.tile_pool(name="xp", bufs=1) as xpool,
            tc.tile_pool(name="wp", bufs=bufs[0]) as wpool,
            tc.tile_pool(name="pp", bufs=bufs[1]) as ppool,
            tc.tile_pool(name="sp", bufs=bufs[2]) as spool,
        ):
            # two persistent x buffers, ping-ponged across blocks
            xb0 = xpool.tile([C, FREE_X], F32, tag="x0", name="xb0")
            xb1 = xpool.tile([C, FREE_X], F32, tag="x1", name="xb1")
            nc.vector.memset(xb0[:], 0.0)
            nc.gpsimd.memset(xb1[:], 0.0)
            xbufs = [xb0, xb1]
            pools = (wpool, ppool, spool)
            # taper the final blocks so the post-DMA compute tail is short
            rbs = [R] * (H // R - 1) + [R // 2, R // 4, R // 4]
            assert sum(rbs) == H

            def body():
                y0 = 0
                for b, rb in enumerate(rbs):
                    x_t = xbufs[b % 2]
                    lo = max(y0 - 1, 0)
                    hi = min(y0 + rb, H - 1)
                    slot0 = lo - (y0 - 1)
                    if b == 0:
                        # slot 0 (row -1) must be zero; stale after iter 1 of
                        # a repeat-timing build, and free to refresh always.
                        nc.vector.memset(x_t[:, 1 : 1 + W], 0.0)
                    if hi == H - 1 and y0 + rb == H:
                        # slot rb+1 (row H) holds stale rows: re-zero first
                        nc.vector.memset(
                            x_t[:, 1 + (rb + 1) * RS : 1 + (rb + 1) * RS + W], 0.0
                        )
                    _emit_block(nc, pools, x_t, x_d, w_d, o_d, y0, rb, (lo, hi, slot0))
                    y0 += rb

            if repeat == 1:
                body()
            else:
                with tc.For_i(0, repeat, 1):
                    body()
    nc.compile()
    return nc


def make_runner(nc):
    """One jitted single-core executable for `nc` (no collectives, no
    partition id). Returns (fn, in_names, out_names, zero_outs); call
    `fn(*inputs, *donated_zero_outs)` with all arrays resident on ONE
    device — execution runs on that device, dispatch is async.

    This deliberately avoids run_bass_kernel_spmd's shard_map path: the
    global concat + per-device dynamic-slice it generates compiles into a
    pathologically large XLA-Neuron program. Independent per-device jits
    sidestep that entirely.
    """
    import jax

    from concourse.bass2jax import (
        _bass_exec_p,
        install_neuronx_cc_hook,
        partition_id_tensor,
    )

    install_neuronx_cc_hook()
    assert not nc.has_collectives
    part_name = nc.partition_id_tensor.name if nc.partition_id_tensor else None
    in_names, out_names, out_avals, zero_outs = [], [], [], []
    for alloc in nc.m.functions[0].allocations:
        if not isinstance(alloc, mybir.MemoryLocationSet):
            continue
        name = alloc.memorylocations[0].name
        if alloc.kind == "ExternalInput":
            if name == part_name:
                continue
            in_names.append(name)
        elif alloc.kind == "ExternalOutput":
            np_dt = mybir.dt.np(alloc.dtype)
            out_avals.append(jax.core.ShapedArray(tuple(alloc.tensor_shape), np_dt))
            out_names.append(name)
            zero_outs.append(np.zeros(tuple(alloc.tensor_shape), np_dt))
    n_params = len(in_names)
    all_in = tuple(
        in_names + out_names + ([part_name] if part_name is not None else [])
    )

    def _body(*args):
        operands = list(args)
        if part_name is not None:
            operands.append(partition_id_tensor())
        return tuple(
            _bass_exec_p.bind(
                *operands,
                out_avals=tuple(out_avals),
                in_names=all_in,
                out_names=tuple(out_names),
                lowering_input_output_aliases=(),
                sim_require_finite=True,
                sim_require_nnan=True,
                nc=nc,
            )
        )

    donate = tuple(range(n_params, n_params + len(out_names)))
    fn = jax.jit(_body, donate_argnums=donate, keep_unused=True)
    return fn, in_names, out_names, zero_outs


_CACHE = {}


def kernel(x: np.ndarray, conv_weights: np.ndarray) -> np.ndarray:
    assert x.shape == (N, C, H, W) and conv_weights.shape == (N, C * 9, H, W)
    import jax

    if "runner" not in _CACHE:
        _CACHE["runner"] = make_runner(build_nc())
    fn, in_names, out_names, zero_outs = _CACHE["runner"]
    devices = jax.devices()[:N]

    futures = []
    for i in range(N):
        per_core = {
            "x": np.ascontiguousarray(x[i], dtype=np.float32),
            "w": np.ascontiguousarray(
                conv_weights[i].reshape(C, 9, H, W), dtype=np.float32
            ),
        }
        args = [jax.device_put(per_core[nm], devices[i]) for nm in in_names]
        args += [jax.device_put(z, devices[i]) for z in zero_outs]
        futures.append(fn(*args))
    outs = [np.asarray(f[0]) for f in futures]
    return np.stack(outs).astype(np.float32)

